# revision 22
# baseline (speedup 1.0000x reference)
"""Multi-head attention (16x1024x768, 12 heads) on 8 Trainium2 cores.

Sharding: pure data-parallel over batch (2 batches per core, no collectives).

Per-core dataflow (all fp32; matmuls issued as float32r = full-rate fp32):
  xT = x^T on PE (identity transpose)                   [768, 1024] per batch
  QT = WqT.T-chunks @ xT  (+bq per-partition on evict)  [768, 1024] feature-major
  KT = likewise                                         [768, 1024]
  V  = xT-chunks.T @ WvT  (+bv via K=1 ones-row matmul) [1024, 768] row-major,
       stored head-interleaved with a ones column -> V_aug [1024, 12, 65]
  per head h:
     ST[j,i]  = KT_h-chunk.T @ QT_h          (K=64 matmul, PSUM [128,1024] per j-chunk)
     PT       = exp(SCALE * ST)              (ACT, PSUM->SBUF)
     OT[65,i] += V_aug_chunk.T @ PT          (row 64 accumulates Z = sum_j exp)
     r        = 1/Z                          (DVE reciprocal, [1,1024])
     rbc      = ones.T @ r                   (K=1 PE broadcast -> [64,1024])
     OcT_h    = OT[0:64] * rbc               (DVE, normalized head output, feature-major)
  Y  = OcT-chunks.T @ WoT (+bo via K=1 ones-row matmul) [1024, 768] -> DMA out
"""

import sys

sys.path.insert(0, "/opt/trn_rl_repo")

import numpy as np

import concourse.bass as bass
import concourse.tile as tile
from concourse import bacc, mybir

FP = mybir.dt.float32
FPR = mybir.dt.float32r

B, N, D = 16, 1024, 768
H, HD = 12, 64
SCALE = HD ** -0.5
NCORES = 8
BPC = B // NCORES  # batches per core
KC = D // 128      # 6 contraction chunks of 128
IC = N // 128      # 8 seq chunks of 128

USE_F32R = True


MMDT = FPR if USE_F32R else FP  # dtype for matmul operands (fp32r = full-rate)


def _halves(total):
    # split a free dim into PSUM-bank-sized matmul chunks (<=512, >=256 for f32r rate)
    out, o = [], 0
    while o < total:
        w = min(512, total - o)
        out.append((o, w))
        o += w
    return out


def build_kernel(loop_reps=1, upto=5):
    import contextlib
    nc = bacc.Bacc("TRN2", target_bir_lowering=False, debug=False)

    x_d = nc.dram_tensor("xs", [BPC * N, D], MMDT, kind="ExternalInput")
    wq_d = nc.dram_tensor("wqT", [D, D], MMDT, kind="ExternalInput")
    wk_d = nc.dram_tensor("wkT", [D, D], MMDT, kind="ExternalInput")
    wv_d = nc.dram_tensor("wvT", [D, D], MMDT, kind="ExternalInput")
    wo_d = nc.dram_tensor("woT", [D, D], MMDT, kind="ExternalInput")
    bq_d = nc.dram_tensor("bqp", [128, KC], FP, kind="ExternalInput")
    bk_d = nc.dram_tensor("bkp", [128, KC], FP, kind="ExternalInput")
    bv_d = nc.dram_tensor("bvr", [1, D], FP, kind="ExternalInput")
    bo_d = nc.dram_tensor("bor", [1, D], FP, kind="ExternalInput")
    id_d = nc.dram_tensor("ident", [128, 128], MMDT, kind="ExternalInput")
    vo_d = nc.dram_tensor("vones", [128, IC * H], MMDT, kind="ExternalInput")
    y_d = nc.dram_tensor("y", [BPC * N, D], FP, kind="ExternalOutput")

    with tile.TileContext(nc) as tc:
        with (
            tc.tile_pool(name="wpool", bufs=1) as wpool,
            tc.tile_pool(name="const", bufs=1) as const,
            tc.tile_pool(name="acts", bufs=1) as acts,
            tc.tile_pool(name="big", bufs=1) as bigp,
            tc.tile_pool(name="xin", bufs=2) as xin,
            tc.tile_pool(name="ptp", bufs=2) as ptp,
            tc.tile_pool(name="small", bufs=1) as smallp,
            tc.tile_pool(name="yout", bufs=2) as yout,
            tc.tile_pool(name="biasbc", bufs=1) as biasbc,
            tc.tile_pool(name="mmps", bufs=2, space="PSUM") as mmps,
            tc.tile_pool(name="otpsum", bufs=2, space="PSUM") as otpsum,
        ):
            # ---- constants / weights ----
            identity = const.tile([128, 128], MMDT)
            nc.gpsimd.dma_start(identity, id_d[:, :])

            w_sb = {}
            for nm, wd, eng in (
                ("q", wq_d, nc.sync),
                ("k", wk_d, nc.sync),
                ("v", wv_d, nc.scalar),
                ("o", wo_d, nc.scalar),
            ):
                wt = wpool.tile([128, KC, D], MMDT, name=f"w{nm}_sb")
                for kc in range(KC):
                    eng.dma_start(wt[:, kc, :], wd[kc * 128:(kc + 1) * 128, :])
                w_sb[nm] = wt
            bq_sb = const.tile([128, KC], FP)
            nc.sync.dma_start(bq_sb, bq_d[:, :])
            bk_sb = const.tile([128, KC], FP)
            nc.sync.dma_start(bk_sb, bk_d[:, :])
            bv_sb = const.tile([1, D], FP)
            nc.sync.dma_start(bv_sb, bv_d[:, :])
            bo_sb = const.tile([1, D], FP)
            nc.sync.dma_start(bo_sb, bo_d[:, :])

            loop_cm = (
                tc.For_i(0, loop_reps) if loop_reps > 1 else contextlib.nullcontext()
            )
            with loop_cm:
              for b in range(BPC):
                row0 = b * N

                # ---- phase 1: load x chunk-wise, transpose on PE -> xT ----
                xT = bigp.tile([128, KC, N], MMDT, tag="big", name=f"xT_{b}")
                for ic in range(IC):
                    xrow = xin.tile([128, D], MMDT, tag="xrow", name=f"xrow_{b}_{ic}")
                    nc.gpsimd.dma_start(
                        xrow, x_d[row0 + ic * 128: row0 + (ic + 1) * 128, :]
                    )
                    for kc in range(KC):
                        tp = mmps.tile([128, 128], MMDT, tag="mm", name=f"tp_{b}_{ic}_{kc}")
                        nc.tensor.transpose(
                            tp, xrow[:, kc * 128:(kc + 1) * 128], identity
                        )
                        nc.vector.tensor_copy(
                            xT[:, kc, ic * 128:(ic + 1) * 128], tp
                        )

                if upto < 2:
                    y_sb0 = yout.tile([128, D], FP, tag="y", name=f"yd_{b}")
                    nc.vector.tensor_copy(y_sb0, xT[:, 0, 0:D])
                    nc.sync.dma_start(y_d[row0:row0 + 128, :], y_sb0)
                    continue
                # ---- phase 2: Q^T and K^T projections (feature-major) ----
                QT = acts.tile([128, KC, N], MMDT, name=f"QT_{b}", tag="QT")
                KT = acts.tile([128, KC, N], MMDT, name=f"KT_{b}", tag="KT")
                for dst, wname, bias in ((QT, "q", bq_sb), (KT, "k", bk_sb)):
                    wt = w_sb[wname]
                    for oc in range(KC):
                        pp = mmps.tile([128, N], FP, tag="mm", name=f"pj_{b}_{wname}_{oc}")
                        for kc in range(KC):
                            for (n0, nw) in _halves(N):
                                nc.tensor.matmul(
                                    pp[:, n0:n0 + nw],
                                    (wt[:, kc, oc * 128:(oc + 1) * 128]),
                                    (xT[:, kc, n0:n0 + nw]),
                                    start=(kc == 0),
                                    stop=(kc == KC - 1),
                                )
                        nc.vector.tensor_scalar_add(
                            dst[:, oc, :], pp, bias[:, oc:oc + 1]
                        )

                if upto < 3:
                    y_sb0 = yout.tile([128, D], FP, tag="y", name=f"yd_{b}")
                    nc.vector.tensor_copy(y_sb0, QT[:, 0, 0:D])
                    nc.sync.dma_start(y_d[row0:row0 + 128, :], y_sb0)
                    continue
                # ---- phase 3: V projection ----
                V = acts.tile([128, IC, H, HD + 1], MMDT, name=f"V_{b}", tag="V")
                nc.gpsimd.dma_start(
                    V[:, :, :, HD:HD + 1].rearrange("p a b c -> p (a b c)"),
                    vo_d[:, :],
                )
                wv = w_sb["v"]
                bvbc = biasbc.tile([128, D], FP, tag="bbc", name=f"bvbc_{b}")
                nc.gpsimd.partition_broadcast(bvbc, bv_sb)
                for ic in range(IC):
                    vp = mmps.tile([128, D], FP, tag="mm", name=f"vp_{b}_{ic}")
                    for (n0, nw) in _halves(D):
                        for kc in range(KC):
                            nc.tensor.matmul(
                                vp[:, n0:n0 + nw],
                                (xT[:, kc, ic * 128:(ic + 1) * 128]),
                                (wv[:, kc, n0:n0 + nw]),
                                start=(kc == 0),
                                stop=(kc == KC - 1),
                            )
                    nc.vector.tensor_tensor(
                        V[:, ic, :, 0:HD],
                        vp.rearrange("p (h d) -> p h d", h=H),
                        bvbc.rearrange("p (h d) -> p h d", h=H),
                        mybir.AluOpType.add,
                    )

                if upto < 4:
                    y_sb0 = yout.tile([128, D], FP, tag="y", name=f"yd_{b}")
                    nc.vector.tensor_copy(y_sb0, QT[:, 0, 0:D])
                    nc.sync.dma_start(y_d[row0:row0 + 128, :], y_sb0)
                    continue
                # ---- phase 4: attention, head by head ----
                OcT = bigp.tile([128, KC, N], MMDT, tag="big", name=f"OcT_{b}")
                for h in range(H):
                    p0 = 64 * (h % 2)
                    c = h // 2
                    ot = otpsum.tile([HD + 1, N], FP, tag="ot", name=f"ot_{b}_{h}")
                    for jc in range(IC):
                        st = mmps.tile([128, N], FP, tag="mm", name=f"st_{b}_{h}_{jc}")
                        for (n0, nw) in _halves(N):
                            nc.tensor.matmul(
                                st[:, n0:n0 + nw],
                                (KT[p0:p0 + 64, c, jc * 128:(jc + 1) * 128]),
                                (QT[p0:p0 + 64, c, n0:n0 + nw]),
                                start=True,
                                stop=True,
                            )
                        pt = ptp.tile([128, N], MMDT, tag="pt", name=f"pt_{b}_{h}_{jc}")
                        nc.scalar.activation(
                            pt, st, mybir.ActivationFunctionType.Exp, scale=SCALE
                        )
                        for (n0, nw) in _halves(N):
                            nc.tensor.matmul(
                                ot[:, n0:n0 + nw],
                                (V[:, jc, h, 0:HD + 1]),
                                (pt[:, n0:n0 + nw]),
                                start=(jc == 0),
                                stop=(jc == IC - 1),
                            )
                    # normalize: OcT_h = OT[0:64] * broadcast(1/Z)
                    r_sb = smallp.tile([1, N], FP, tag="r", name=f"r_{b}_{h}")
                    nc.vector.reciprocal(r_sb, ot[HD:HD + 1, :])
                    rbc_sb = smallp.tile([64, N], FP, tag="rbc", name=f"rbcs_{b}_{h}")
                    nc.gpsimd.partition_broadcast(rbc_sb, r_sb)
                    nc.vector.tensor_tensor(
                        OcT[p0:p0 + 64, c, :],
                        ot[0:HD, :],
                        rbc_sb,
                        mybir.AluOpType.mult,
                    )

                if upto < 5:
                    y_sb0 = yout.tile([128, D], FP, tag="y", name=f"yd_{b}")
                    nc.vector.tensor_copy(y_sb0, OcT[:, 0, 0:D])
                    nc.sync.dma_start(y_d[row0:row0 + 128, :], y_sb0)
                    continue
                # ---- phase 5: output projection ----
                wo = w_sb["o"]
                bobc = biasbc.tile([128, D], FP, tag="bbc", name=f"bobc_{b}")
                nc.gpsimd.partition_broadcast(bobc, bo_sb)
                for ic in range(IC):
                    yp = mmps.tile([128, D], FP, tag="mm", name=f"yp_{b}_{ic}")
                    for (n0, nw) in _halves(D):
                        for kc in range(KC):
                            nc.tensor.matmul(
                                yp[:, n0:n0 + nw],
                                (OcT[:, kc, ic * 128:(ic + 1) * 128]),
                                (wo[:, kc, n0:n0 + nw]),
                                start=(kc == 0),
                                stop=(kc == KC - 1),
                            )
                    y_sb = yout.tile([128, D], FP, tag="y", name=f"y_{b}_{ic}")
                    nc.vector.tensor_tensor(y_sb, yp, bobc, mybir.AluOpType.add)
                    nc.sync.dma_start(
                        y_d[row0 + ic * 128: row0 + (ic + 1) * 128, :], y_sb
                    )

    nc.compile()
    return nc


_CACHE = {}

TRACE = False
LAST_RESULTS = None


def _get_nc(loop_reps=1, upto=5):
    key = ("nc", loop_reps, upto)
    if key not in _CACHE:
        _CACHE[key] = build_kernel(loop_reps, upto)
    return _CACHE[key]


def _get_runner(loop_reps=1, upto=5):
    """Build (once) a persistently-cached jitted shard_map executable.

    Mirrors concourse.bass2jax.run_bass_via_pjrt but keeps the jitted
    callable alive across kernel() calls so repeat calls skip retracing,
    XLA compile, and NEFF reload.
    """
    if ("runner", loop_reps, upto) in _CACHE:
        return _CACHE[("runner", loop_reps, upto)]

    import jax
    from jax.experimental.shard_map import shard_map
    from jax.sharding import Mesh, PartitionSpec
    from concourse import mybir as _mybir
    from concourse.bass2jax import (
        _bass_exec_p,
        install_neuronx_cc_hook,
        partition_id_tensor,
    )

    nc = _get_nc(loop_reps, upto)
    install_neuronx_cc_hook()

    pid_name = nc.partition_id_tensor.name if nc.partition_id_tensor else None
    in_names, out_names, out_avals = [], [], []
    for alloc in nc.m.functions[0].allocations:
        if not isinstance(alloc, _mybir.MemoryLocationSet):
            continue
        name = alloc.memorylocations[0].name
        if alloc.kind == "ExternalInput":
            if name == pid_name:
                continue
            in_names.append(name)
        elif alloc.kind == "ExternalOutput":
            out_names.append(name)
            out_avals.append(
                jax.core.ShapedArray(
                    tuple(alloc.tensor_shape), _mybir.dt.np(alloc.dtype)
                )
            )
    n_params = len(in_names)
    n_outs = len(out_names)
    all_names = in_names + out_names
    if pid_name is not None:
        all_names = all_names + [pid_name]

    def _body(*args):
        operands = list(args)
        if pid_name is not None:
            operands.append(partition_id_tensor())
        outs = _bass_exec_p.bind(
            *operands,
            out_avals=tuple(out_avals),
            in_names=tuple(all_names),
            out_names=tuple(out_names),
            lowering_input_output_aliases=(),
            sim_require_finite=True,
            sim_require_nnan=True,
            nc=nc,
        )
        return tuple(outs)

    devices = jax.devices()[:NCORES]
    mesh = Mesh(np.asarray(devices), ("core",))
    donate = tuple(range(n_params, n_params + n_outs))
    sharded = jax.jit(
        shard_map(
            _body,
            mesh=mesh,
            in_specs=(PartitionSpec("core"),) * (n_params + n_outs),
            out_specs=(PartitionSpec("core"),) * n_outs,
            check_rep=False,
        ),
        donate_argnums=donate,
        keep_unused=True,
    )
    _CACHE[("runner", loop_reps, upto)] = (
        sharded, in_names, out_names, out_avals, n_params
    )
    return _CACHE[("runner", loop_reps, upto)]


def run_on_cores(in_maps):
    """Run the SPMD kernel with a cached executable; returns list of out dicts."""
    import jax
    import jax.numpy as jnp

    sharded, in_names, out_names, out_avals, n_params = _get_runner()
    concat_in = [
        np.concatenate([np.asarray(m[name]) for m in in_maps], axis=0)
        for name in in_names
    ]
    zeros = [
        jnp.zeros((NCORES * a.shape[0], *a.shape[1:]), a.dtype) for a in out_avals
    ]
    outs = sharded(*concat_in, *zeros)
    outs = [np.asarray(o) for o in outs]
    return [
        {
            name: outs[i].reshape(NCORES, *out_avals[i].shape)[c]
            for i, name in enumerate(out_names)
        }
        for c in range(NCORES)
    ]


def make_in_maps(x, Wq, bq, Wk, bk, Wv, bv, Wo, bo):
    shared = {
        "wqT": np.ascontiguousarray(np.asarray(Wq, np.float32).T),
        "wkT": np.ascontiguousarray(np.asarray(Wk, np.float32).T),
        "wvT": np.ascontiguousarray(np.asarray(Wv, np.float32).T),
        "woT": np.ascontiguousarray(np.asarray(Wo, np.float32).T),
        "bqp": np.ascontiguousarray(np.asarray(bq, np.float32).reshape(KC, 128).T),
        "bkp": np.ascontiguousarray(np.asarray(bk, np.float32).reshape(KC, 128).T),
        "bvr": np.asarray(bv, np.float32).reshape(1, D).copy(),
        "bor": np.asarray(bo, np.float32).reshape(1, D).copy(),
        "ident": np.eye(128, dtype=np.float32),
        "vones": np.ones((128, IC * H), np.float32),
    }
    x = np.asarray(x, np.float32)
    in_maps = []
    for core in range(NCORES):
        m = dict(shared)
        m["xs"] = np.ascontiguousarray(
            x[core * BPC:(core + 1) * BPC].reshape(BPC * N, D)
        )
        in_maps.append(m)
    return in_maps


def kernel(x, Wq, bq, Wk, bk, Wv, bv, Wo, bo):
    import time

    in_maps = make_in_maps(x, Wq, bq, Wk, bk, Wv, bv, Wo, bo)
    last_err = None
    for attempt in range(3):
        try:
            results = run_on_cores(in_maps)
            break
        except Exception as e:  # transient device wedges recover on retry
            last_err = e
            if "UNRECOVERABLE" not in str(e) and "UNAVAILABLE" not in str(e):
                raise
            time.sleep(5.0)
    else:
        raise last_err
    y = np.concatenate(
        [results[c]["y"].reshape(BPC, N, D) for c in range(NCORES)], axis=0
    )
    return y


def bench(x, Wq, bq, Wk, bk, Wv, bv, Wo, bo, reps=20, loop_reps=1, upto=5):
    """Time repeated device executions with device-resident inputs."""
    import time
    import jax
    import jax.numpy as jnp

    in_maps = make_in_maps(x, Wq, bq, Wk, bk, Wv, bv, Wo, bo)
    sharded, in_names, out_names, out_avals, n_params = _get_runner(loop_reps, upto)
    concat_in = [
        np.concatenate([np.asarray(m[name]) for m in in_maps], axis=0)
        for name in in_names
    ]
    dev_in = [jax.device_put(a) for a in concat_in]

    def zeros():
        return [
            jnp.zeros((NCORES * a.shape[0], *a.shape[1:]), a.dtype)
            for a in out_avals
        ]

    # warmup
    out = sharded(*dev_in, *zeros())
    jax.block_until_ready(out)
    times = []
    for _ in range(reps):
        z = zeros()
        jax.block_until_ready(z)
        t0 = time.perf_counter()
        out = sharded(*dev_in, *z)
        jax.block_until_ready(out)
        times.append(time.perf_counter() - t0)
    return times
